# revision 30
# baseline (speedup 1.0000x reference)
"""Causal self-attention (RoPE, 16 heads, d=64, B=4, T=2048, C=1024) on 8 TRN2 cores.

Sharding: core g = (batch b = g//2, head-group hg = g%2 covering 8 heads).
Data-parallel over B, tensor-parallel over heads.  Each core computes the
partial out-projection (its 8 heads' contribution, no bias); the host sums
the two head-group partials per batch and adds b_out.

Per-core kernel (all matmul operands bf16, fp32 PSUM accumulation), emitted
as one interleaved stream per 512-wide t-window so the PE never starves:

  qkv segment I (dripped into attention window I-1 as PE gap-filler):
    q/k/v = xT.T @ Wqkv (xT pre-transposed on host as the stationary
    operand); RoPE on q,k in natural [t, d] layout on DVE (cos/sin muls
    against compact [T,64] tables via stride-0 broadcast APs); q,k
    HW-DMA-transposed into [d, t] layout (Sync queue carries ONLY
    transposes so the xbar never flips modes); v copied into [V | ones64]
    stationary tiles.  Input loads alternate between the GpSimd SWDGE
    queue and the Scalar HWDGE queue in consumption-order slices so the
    first qkv matmuls start ~2us in, chasing the loads.  Segment 0 (which
    runs before any attention) cycles its qkv PSUM tiles across the
    still-free attention banks for a 4-deep pipeline and sends its
    v-copies to the then-idle ACT engine, so the PE is not WAR-throttled
    behind DVE's rope reads while it chases the loads.

  attention window I, per head-pair:
    S^T[s,t] = k^T q with both heads packed in the PE array via
    tile_position row tiling (contraction dim is only 64); causal upper
    blocks skipped; exp on ACT (scale=1/8, padding mask as per-partition
    bias, no max subtraction -- logits are ~N(0,1)); the diagonal block's
    upper triangle is zeroed AFTER the exp by a cheap bf16 2x-mode DVE
    multiply with a 0/1 mask (no PSUM seeding matmuls);
    AV with [V | ones64] stationary and exp(S^T) streaming accumulates
    attn_out^T[d,t] on psT[0:64] and the denominator replicated on
    psT[64:128]; softmax division = both heads' denominators staged to
    one SBUF tile (custom DVE ops read matmul-written PSUM as garbage),
    one reciprocal_approx_fast over [128,512] (~5x faster than the
    iterative divide), and two DVE multiplies straight off PSUM into the
    out-projection's stationary layout (no attention-output transpose);
    out-projection units are deferred into the late ACT-bound windows
    where the PE otherwise idles behind the exp stream; "heater" matmuls
    keep the PE's HAM clock at 2.4 GHz when the drip runs dry.

  Output is stored bf16 (halves the store traffic); the host sums the two
  head-group partials in fp32 and adds b_out.
"""

import os
from contextlib import ExitStack

import numpy as np
import ml_dtypes

B, T, C = 4, 2048, 1024
H, D = 16, 64
HG = 8            # heads per core
NCORES = 8
TB = T // 128     # 16 t/s-blocks of 128
CBN = C // 128    # 8 contraction chunks
NP = HG // 2      # 4 head pairs
NI = T // 512     # 4 t-windows of 512
ROPE_BASE = 10000.0

_PROG = None
_LAST_RESULTS = None


def _build_program():
    import concourse.bass as bass
    import concourse.tile as tile
    from concourse import bacc, mybir

    f32 = mybir.dt.float32
    bf = mybir.dt.bfloat16
    EXP = mybir.ActivationFunctionType.Exp
    CPY = mybir.ActivationFunctionType.Copy

    nc = bacc.Bacc("TRN2", target_bir_lowering=False, debug=False)

    xT = nc.dram_tensor("xT", [C, T], bf, kind="ExternalInput").ap()
    wqkv = nc.dram_tensor("wqkv", [C, 3 * HG * D], bf, kind="ExternalInput").ap()
    wout = nc.dram_tensor("wout", [HG * D, C], bf, kind="ExternalInput").ap()
    cosT = nc.dram_tensor("cosT", [T, D], bf, kind="ExternalInput").ap()
    sinT = nc.dram_tensor("sinT", [T, D], bf, kind="ExternalInput").ap()
    padb = nc.dram_tensor("padb", [128, TB], f32, kind="ExternalInput").ap()
    tri = nc.dram_tensor("tri", [128, 128], bf, kind="ExternalInput").ap()
    outp = nc.dram_tensor("outp", [T, C], bf, kind="ExternalOutput").ap()

    with tile.TileContext(nc) as tc, ExitStack() as ctx:
        singles = ctx.enter_context(tc.tile_pool(name="singles", bufs=1))

        # ---- global SBUF tensors.  Input loads alternate between the
        # GpSimd SWDGE queue and the Scalar HWDGE queue (2x load bandwidth;
        # the Sync queue is reserved for DMA transposes so the xbar never
        # flips modes).  Order: (xt0,w0),(xt1,w1) so the first qkv matmuls
        # start ~5us in, then cos/sin (first RoPE), then the rest.
        xt_sb = [singles.tile([128, T], bf, name=f"xt{cb}", tag=f"xt{cb}")
                 for cb in range(CBN)]
        w_sb = [singles.tile([128, 3 * HG * D], bf, name=f"w{cb}", tag=f"w{cb}")
                for cb in range(CBN)]
        wo_sb = [singles.tile([128, C], bf, name=f"wo{c}", tag=f"wo{c}")
                 for c in range(4)]
        cos_sb = singles.tile([128, TB, D], bf, name="cos_sb", tag="cos_sb")
        sin_sb = singles.tile([128, TB, D], bf, name="sin_sb", tag="sin_sb")
        padb_sb = singles.tile([128, TB], f32, name="padb_sb", tag="padb_sb")
        tri_sb = singles.tile([128, 128], bf, name="tri_sb", tag="tri_sb")

        # Load order is pipelined against segment 0's consumption: the
        # first qkv matmul only needs xt[cb][:, 0:512] (covers tb 0-3) and
        # the q-columns of w[cb], so those 256KB slices go first and the PE
        # starts ~1.5us in, chasing the loads.  k/v weight columns, rope
        # tables, the t>=512 remainder of xT, and wout follow in
        # consumption order.
        qs = [nc.gpsimd, nc.scalar]
        loads = []
        for cb in range(CBN):
            loads.append((xt_sb[cb][:, 0:512], xT[cb * 128:(cb + 1) * 128, 0:512]))
            loads.append((w_sb[cb][:, 0:512], wqkv[cb * 128:(cb + 1) * 128, 0:512]))
        loads.append((cos_sb, cosT.rearrange("(tb p) d -> p tb d", p=128)))
        loads.append((sin_sb, sinT.rearrange("(tb p) d -> p tb d", p=128)))
        for cb in range(CBN):
            loads.append((w_sb[cb][:, 512:1024],
                          wqkv[cb * 128:(cb + 1) * 128, 512:1024]))
        for cb in range(CBN):
            loads.append((w_sb[cb][:, 1024:1536],
                          wqkv[cb * 128:(cb + 1) * 128, 1024:1536]))
        loads.append((padb_sb, padb))
        loads.append((tri_sb, tri))
        for cb in range(CBN):
            loads.append((xt_sb[cb][:, 512:T], xT[cb * 128:(cb + 1) * 128, 512:T]))
        for c in range(4):
            loads.append((wo_sb[c], wout[c * 128:(c + 1) * 128, :]))
        for i, (dst, src) in enumerate(loads):
            qs[i % 2].dma_start(out=dst, in_=src)

        # q^T/k^T: [within-pair col (head-lo d / head-hi d), s-block, pair, t]
        qT_all = singles.tile([128, TB, NP, 128], bf, name="qT_all", tag="qT_all")
        kT_all = singles.tile([128, TB, NP, 128], bf, name="kT_all", tag="kT_all")
        # v with 64 ones columns per head: the AV matmul then emits the
        # softmax denominator replicated on 64 partitions (rows 64-127),
        # so the reciprocal runs wide instead of a 1-partition crawl
        vones = singles.tile([128, TB, HG, 128], bf, name="vones", tag="vones")
        # only window 0's ones blocks up front (the full-tensor memset was
        # 6.9us of DVE squarely on the startup critical chain); each later
        # segment memsets its own blocks as part of the drip
        nc.vector.memset(vones[:, 0:4, :, D:128], 1.0)

        # qkv + attention interleaved per 512-wide t-window so the PE stream
        # stays dense (HAM stays at 2.4 GHz): attention for window I only
        # needs q/k/v blocks 0..4I+3, which segment I of the qkv loop topped
        # off.  One shared PSUM pool: qkv 2 + sAB 2x2 + psT 2 = 8 banks.
        with tc.tile_pool(name="psum", bufs=2, space="PSUM") as psum, \
             tc.tile_pool(name="rope", bufs=4) as rope_pool, \
             tc.tile_pool(name="qknat", bufs=3) as qk_pool, \
             tc.tile_pool(name="exps", bufs=3) as exp_pool, \
             tc.tile_pool(name="attnT", bufs=4) as aT_pool, \
             tc.tile_pool(name="recips", bufs=2) as rc_pool, \
             tc.tile_pool(name="outsb", bufs=2) as out_pool:
            def qkv_segment(I):
                """Generator emitting segment I's qkv matmuls in half-tensor
                chunks (yield points), so the caller can drip them into the
                attention loop as PE gap-filler while ACT grinds exps."""
                if I > 0:
                    nc.vector.memset(vones[:, 4 * I:4 * I + 4, :, D:128], 1.0)
                # Segment 0 runs before the attention windows, so the sAB/
                # psT PSUM banks are still free: cycle its qkv tiles across
                # them for a 4-deep pipeline (the 2-buffer qkv rotation
                # otherwise WAR-stalls the PE behind DVE's rope reads of the
                # previous-but-one unit).  Its v-copies go to the then-idle
                # ACT engine for the same reason.
                tags = ("qkv", "avA", "qkv", "avB") if I == 0 else ("qkv",)
                u = 0
                for tb in range(4 * I, 4 * I + 4):
                    for which, base in (("q", 0), ("k", 512), ("v", 1024)):
                        tag = tags[u % len(tags)]
                        u += 1
                        ps = psum.tile([128, HG, D], f32, name=f"ps{which}",
                                       tag=tag, bufs=(2 if tag == "qkv" else 1))
                        for cb in range(CBN):
                            nc.tensor.matmul(
                                ps, xt_sb[cb][:, tb * 128:(tb + 1) * 128],
                                w_sb[cb][:, base:base + 512],
                                start=(cb == 0), stop=(cb == CBN - 1))
                            if cb == 3:
                                yield
                        if which == "v":
                            if I == 0:
                                nc.scalar.activation(
                                    out=vones[:, tb, :, 0:D], in_=ps, func=CPY)
                            else:
                                nc.vector.tensor_copy(
                                    out=vones[:, tb, :, 0:D], in_=ps)
                            yield
                            continue
                        # rope: P_c = qkv*cosF, P_s = qkv*sinF (compact [T,64]
                        # tables broadcast over the 8 heads; both halves of
                        # each head carry the same table value), then
                        # lo = P_c.lo - P_s.hi ; hi = P_s.lo + P_c.hi
                        cosb = cos_sb[:, tb].unsqueeze(1).broadcast_to((128, HG, D))
                        sinb = sin_sb[:, tb].unsqueeze(1).broadcast_to((128, HG, D))
                        pc = rope_pool.tile([128, HG, D], f32, name="pc", tag="rt")
                        psn = rope_pool.tile([128, HG, D], f32, name="psn", tag="rt")
                        nc.vector.tensor_mul(pc, ps, cosb)
                        nc.vector.tensor_mul(psn, ps, sinb)
                        ro = qk_pool.tile([128, HG, D], bf, name="ro", tag="ro")
                        nc.vector.tensor_sub(
                            ro[:, :, 0:32], pc[:, :, 0:32], psn[:, :, 32:64])
                        nc.vector.tensor_add(
                            ro[:, :, 32:64], psn[:, :, 0:32], pc[:, :, 32:64])
                        dst = qT_all if which == "q" else kT_all
                        nc.sync.dma_start_transpose(out=dst[:, tb, :, :], in_=ro)
                        yield

            pending_out = []
            for I in range(NI):
                if I == 0:
                    for _ in qkv_segment(0):
                        pass
                nxt = qkv_segment(I + 1) if I + 1 < NI else None
                n_chunks = 4 * 3 * 3  # yield points per segment
                n_iters = 4 * (4 * I + 4)
                emitted = it = 0
                # out-projection pop budget per window: defer most of it into
                # the late, ACT-bound windows where the PE otherwise idles
                # behind the exp stream
                pops_left = (0, 4, 8, 10 ** 9)[I]

                def drip():
                    nonlocal emitted
                    got = 0
                    if nxt is None:
                        return got
                    # finish the whole segment by ~75% of the window so the
                    # next window's S matmuls never wait on rope/transposes
                    due = (4 * it * n_chunks) // (3 * n_iters)
                    while emitted < due:
                        if next(nxt, "done") == "done":
                            break
                        emitted += 1
                        got += 1
                    return got

                # ---- attention window I ----
                aT_I = aT_pool.tile([128, NP, 512], bf, name="aT_I", tag="aT_I")
                for p in range(NP):
                    psTA = psum.tile([128, 512], f32, name="psTA", tag="avA", bufs=1)
                    psTB = psum.tile([128, 512], f32, name="psTB", tag="avB", bufs=1)

                    def emit_av(j, eAB):
                        off = max(j - 4 * I, 0) * 128
                        for h2, psT in ((0, psTA), (1, psTB)):
                            nc.tensor.matmul(
                                psT[:, off:512],
                                vones[:, j, 2 * p + h2, :],
                                eAB[:, h2, off:512],
                                start=(j == 0), stop=(j == 4 * I + 3))

                    prev = None
                    dry = False
                    for j in range(4 * I + 4):
                        jl = j - 4 * I
                        off = max(jl, 0) * 128
                        sAB = psum.tile([128, 2, 512], f32, name="sAB", tag="sAB", bufs=2)
                        if dry:
                            # "heater" matmul: PE would otherwise sit ~50%
                            # idle behind ACT and HAM-downclock to 1.2 GHz;
                            # burn a throwaway matmul into the bank the next
                            # S matmul overwrites anyway
                            nc.tensor.matmul(
                                sAB[:, 0, :], xt_sb[0][:, 0:128], xt_sb[0][:, 0:512],
                                start=True, stop=True, skip_group_check=True)
                        HALVES = ((0, slice(0, 64)), (1, slice(64, 128)))
                        if jl >= 0:
                            # keep the row-tiled pair adjacent so the two
                            # heads overlap in the PE array
                            for h2, rows in HALVES:
                                nc.tensor.matmul(
                                    sAB[:, h2, off:off + 128],
                                    kT_all[rows, j, p, :],
                                    qT_all[rows, 4 * I + jl, p, :],
                                    start=True, stop=True,
                                    tile_position=(h2 * 64, 0),
                                    skip_group_check=True)
                            if off + 128 < 512:
                                for h2, rows in HALVES:
                                    nc.tensor.matmul(
                                        sAB[:, h2, off + 128:512],
                                        kT_all[rows, j, p, :],
                                        qT_all[rows, 4 * I + jl + 1:4 * I + 4, p, :],
                                        start=True, stop=True,
                                        tile_position=(h2 * 64, 0))
                        else:
                            for h2, rows in HALVES:
                                nc.tensor.matmul(
                                    sAB[:, h2, :],
                                    kT_all[rows, j, p, :],
                                    qT_all[rows, 4 * I:4 * I + 4, p, :],
                                    start=True, stop=True,
                                    tile_position=(h2 * 64, 0))
                        eAB = exp_pool.tile([128, 2, 512], bf, name="eAB", tag="eAB")
                        bias = padb_sb[:, j:j + 1]
                        nc.scalar.activation(
                            out=eAB[:, :, off:512], in_=sAB[:, :, off:512],
                            func=EXP, bias=bias, scale=0.125)
                        if jl >= 0:
                            # zero the diagonal block's upper triangle (keys
                            # after the query) post-exp: bf16 2x-mode DVE
                            # multiply with the 0/1 mask, replacing the PSUM
                            # -1e30 seeding matmuls
                            trib = tri_sb.unsqueeze(1).broadcast_to((128, 2, 128))
                            nc.vector.tensor_mul(
                                eAB[:, :, off:off + 128],
                                eAB[:, :, off:off + 128], trib)
                        if prev is not None:
                            emit_av(*prev)
                        prev = (j, eAB)
                        it += 1
                        did_out = False
                        if pending_out and pops_left > 0:
                            pending_out.pop(0)()
                            pops_left -= 1
                            did_out = True
                        dry = drip() == 0 and not did_out and (it % 2 == 0)
                    emit_av(*prev)

                    # softmax normalization: stage both heads' replicated
                    # denominators (psT[64:128]) into one SBUF tile, one
                    # reciprocal_approx_fast over [128,512] (custom DVE op,
                    # ~5x faster than the iterative divide -- but it reads
                    # matmul-written PSUM as garbage, hence the SBUF staging
                    # copies), then multiply the numerators straight off
                    # PSUM into the out-projection's stationary layout.
                    cpd = rc_pool.tile([128, 512], f32, name="cpd", tag="cpd")
                    nc.vector.tensor_copy(out=cpd[0:64, :], in_=psTA[D:128, :])
                    nc.vector.tensor_copy(out=cpd[64:128, :], in_=psTB[D:128, :])
                    rc = rc_pool.tile([128, 512], f32, name="rc", tag="rc")
                    nc.vector.reciprocal_approx_fast(rc, cpd)
                    nc.vector.tensor_mul(
                        aT_I[0:64, p, :], psTA[0:D, :], rc[0:64, :])
                    nc.vector.tensor_mul(
                        aT_I[64:128, p, :], psTB[0:D, :], rc[64:128, :])

                # out-projection units are deferred into the next window's
                # loop as more PE gap-filler
                def make_out_unit(aT, i, il, n):
                    def emit():
                        pso = psum.tile([128, 512], f32, name="pso", tag="sAB", bufs=2)
                        for c in range(4):
                            nc.tensor.matmul(
                                pso,
                                aT[:, c, il * 128:(il + 1) * 128],
                                wo_sb[c][:, n * 512:(n + 1) * 512],
                                start=(c == 0), stop=(c == 3))
                        osb = out_pool.tile([128, 512], bf, name="osb", tag="osb")
                        nc.vector.tensor_copy(out=osb, in_=pso)
                        nc.gpsimd.dma_start(
                            out=outp[i * 128:(i + 1) * 128, n * 512:(n + 1) * 512],
                            in_=osb)
                    return emit
                for il in range(4):
                    for n in range(2):
                        pending_out.append(make_out_unit(aT_I, 4 * I + il, il, n))
                if nxt is not None:
                    for _ in nxt:
                        pass
            for f in pending_out:
                f()

    nc.compile()
    return nc


def _get_program():
    global _PROG
    if _PROG is None:
        _PROG = _build_program()
    return _PROG


def _rope_tables():
    bf16 = ml_dtypes.bfloat16
    inv = 1.0 / (ROPE_BASE ** (np.arange(0, D, 2, dtype=np.float64) / D))
    f = np.arange(T, dtype=np.float64)[:, None] * inv[None, :]  # [T, 32]
    c = np.cos(f)
    s = np.sin(f)
    # both 32-col halves carry the same table value
    cosT = np.concatenate([c, c], axis=1).astype(bf16)  # [T, 64]
    sinT = np.concatenate([s, s], axis=1).astype(bf16)
    return cosT, sinT


def kernel(x, attention_mask, W_qkv, W_out, b_out):
    global _LAST_RESULTS
    from concourse.bass_utils import run_bass_kernel_spmd

    nc = _get_program()
    bf16 = ml_dtypes.bfloat16
    x = np.asarray(x, dtype=np.float32)
    attention_mask = np.asarray(attention_mask)
    W_qkv = np.asarray(W_qkv, dtype=np.float32)
    W_out = np.asarray(W_out, dtype=np.float32)
    b_out = np.asarray(b_out, dtype=np.float32)

    cosT, sinT = _rope_tables()
    # causal mask for the diagonal block: keep key s <= query t
    tri = np.where(np.arange(128)[:, None] <= np.arange(128)[None, :], 1.0, 0.0)
    tri = tri.astype(bf16)

    in_maps = []
    for g in range(NCORES):
        b, hg = g // 2, g % 2
        sl = slice(hg * 512, hg * 512 + 512)
        wq = W_qkv[:, 0 * C:][:, sl]
        wk = W_qkv[:, 1 * C:2 * C][:, sl]
        wv = W_qkv[:, 2 * C:3 * C][:, sl]
        wqkv_g = np.ascontiguousarray(
            np.concatenate([wq, wk, wv], axis=1)).astype(bf16)
        xT_g = np.ascontiguousarray(x[b].T).astype(bf16)
        wout_g = np.ascontiguousarray(W_out[sl, :]).astype(bf16)
        padb_g = np.ascontiguousarray(
            np.where(attention_mask[b] != 0, 0.0, -1e30)
            .astype(np.float32).reshape(TB, 128).T)
        in_maps.append({
            "xT": xT_g, "wqkv": wqkv_g, "wout": wout_g,
            "cosT": cosT, "sinT": sinT, "padb": padb_g, "tri": tri,
        })

    res = run_bass_kernel_spmd(nc, in_maps, list(range(NCORES)))
    _LAST_RESULTS = res
    out = np.empty((B, T, C), dtype=np.float32)
    for b in range(B):
        out[b] = (res.results[2 * b]["outp"].astype(np.float32)
                  + res.results[2 * b + 1]["outp"].astype(np.float32) + b_out)
    return out


# revision 32
# speedup vs baseline: 7604.5246x; 7604.5246x over previous
"""Causal self-attention (RoPE, 16 heads, d=64, B=4, T=2048, C=1024) on 8 TRN2 cores.

Sharding: core g = (batch b = g//2, head-group hg = g%2 covering 8 heads).
Data-parallel over B, tensor-parallel over heads.  Each core computes the
partial out-projection (its 8 heads' contribution, no bias); the host sums
the two head-group partials per batch and adds b_out.

Per-core kernel (all matmul operands bf16, fp32 PSUM accumulation), emitted
as one interleaved stream per 512-wide t-window so the PE never starves:

  qkv segment I (dripped into attention window I-1 as PE gap-filler):
    q/k/v = xT.T @ Wqkv (xT pre-transposed on host as the stationary
    operand); RoPE on q,k in natural [t, d] layout on DVE (cos/sin muls
    against compact [T,64] tables via stride-0 broadcast APs); q,k
    HW-DMA-transposed into [d, t] layout (Sync queue carries ONLY
    transposes so the xbar never flips modes); v copied into [V | ones64]
    stationary tiles.  Input loads alternate between the GpSimd SWDGE
    queue and the Scalar HWDGE queue in consumption-order slices so the
    first qkv matmuls start ~2us in, chasing the loads.  Segment 0 (which
    runs before any attention) cycles its qkv PSUM tiles across the
    still-free attention banks for a 4-deep pipeline and sends its
    v-copies to the then-idle ACT engine, so the PE is not WAR-throttled
    behind DVE's rope reads while it chases the loads.

  attention window I, per head-pair:
    S^T[s,t] = k^T q with both heads packed in the PE array via
    tile_position row tiling (contraction dim is only 64); causal upper
    blocks skipped; exp on ACT (scale=1/8, padding mask as per-partition
    bias, no max subtraction -- logits are ~N(0,1)); the diagonal block's
    upper triangle is zeroed AFTER the exp by a cheap bf16 2x-mode DVE
    multiply with a 0/1 mask (no PSUM seeding matmuls);
    AV with [V | ones64] stationary and exp(S^T) streaming accumulates
    attn_out^T[d,t] on psT[0:64] and the denominator replicated on
    psT[64:128]; softmax division = both heads' denominators staged to
    one SBUF tile (custom DVE ops read matmul-written PSUM as garbage),
    one reciprocal_approx_fast over [128,512] (~5x faster than the
    iterative divide), and two DVE multiplies straight off PSUM into the
    out-projection's stationary layout (no attention-output transpose);
    out-projection units are deferred into the late ACT-bound windows
    where the PE otherwise idles behind the exp stream; "heater" matmuls
    keep the PE's HAM clock at 2.4 GHz when the drip runs dry.

  Output is stored bf16 (halves the store traffic); the host sums the two
  head-group partials in fp32 and adds b_out.
"""

import os
from contextlib import ExitStack

import numpy as np
import ml_dtypes

B, T, C = 4, 2048, 1024
H, D = 16, 64
HG = 8            # heads per core
NCORES = 8
TB = T // 128     # 16 t/s-blocks of 128
CBN = C // 128    # 8 contraction chunks
NP = HG // 2      # 4 head pairs
NI = T // 512     # 4 t-windows of 512
ROPE_BASE = 10000.0

_PROG = None
_LAST_RESULTS = None


def _build_program():
    import concourse.bass as bass
    import concourse.tile as tile
    from concourse import bacc, mybir

    f32 = mybir.dt.float32
    bf = mybir.dt.bfloat16
    EXP = mybir.ActivationFunctionType.Exp
    CPY = mybir.ActivationFunctionType.Copy

    nc = bacc.Bacc("TRN2", target_bir_lowering=False, debug=False)

    xT = nc.dram_tensor("xT", [C, T], bf, kind="ExternalInput").ap()
    wqkv = nc.dram_tensor("wqkv", [C, 3 * HG * D], bf, kind="ExternalInput").ap()
    wout = nc.dram_tensor("wout", [HG * D, C], bf, kind="ExternalInput").ap()
    cosT = nc.dram_tensor("cosT", [T, D], bf, kind="ExternalInput").ap()
    sinT = nc.dram_tensor("sinT", [T, D], bf, kind="ExternalInput").ap()
    padb = nc.dram_tensor("padb", [128, TB], f32, kind="ExternalInput").ap()
    tri = nc.dram_tensor("tri", [128, 128], bf, kind="ExternalInput").ap()
    outp = nc.dram_tensor("outp", [T, C], bf, kind="ExternalOutput").ap()

    with tile.TileContext(nc) as tc, ExitStack() as ctx:
        singles = ctx.enter_context(tc.tile_pool(name="singles", bufs=1))

        # ---- global SBUF tensors.  Input loads alternate between the
        # GpSimd SWDGE queue and the Scalar HWDGE queue (2x load bandwidth;
        # the Sync queue is reserved for DMA transposes so the xbar never
        # flips modes).  Order: (xt0,w0),(xt1,w1) so the first qkv matmuls
        # start ~5us in, then cos/sin (first RoPE), then the rest.
        xt_sb = [singles.tile([128, T], bf, name=f"xt{cb}", tag=f"xt{cb}")
                 for cb in range(CBN)]
        w_sb = [singles.tile([128, 3 * HG * D], bf, name=f"w{cb}", tag=f"w{cb}")
                for cb in range(CBN)]
        wo_sb = [singles.tile([128, C], bf, name=f"wo{c}", tag=f"wo{c}")
                 for c in range(4)]
        cos_sb = singles.tile([128, TB, D], bf, name="cos_sb", tag="cos_sb")
        sin_sb = singles.tile([128, TB, D], bf, name="sin_sb", tag="sin_sb")
        padb_sb = singles.tile([128, TB], f32, name="padb_sb", tag="padb_sb")
        tri_sb = singles.tile([128, 128], bf, name="tri_sb", tag="tri_sb")

        # Load order is pipelined against segment 0's consumption: the
        # first qkv matmul only needs xt[cb][:, 0:512] (covers tb 0-3) and
        # the q-columns of w[cb], so those 256KB slices go first and the PE
        # starts ~1.5us in, chasing the loads.  k/v weight columns, rope
        # tables, the t>=512 remainder of xT, and wout follow in
        # consumption order.
        qs = [nc.gpsimd, nc.scalar]
        loads = []
        for cb in range(CBN):
            loads.append((xt_sb[cb][:, 0:512], xT[cb * 128:(cb + 1) * 128, 0:512]))
            loads.append((w_sb[cb][:, 0:512], wqkv[cb * 128:(cb + 1) * 128, 0:512]))
        loads.append((cos_sb, cosT.rearrange("(tb p) d -> p tb d", p=128)))
        loads.append((sin_sb, sinT.rearrange("(tb p) d -> p tb d", p=128)))
        for cb in range(CBN):
            loads.append((w_sb[cb][:, 512:1024],
                          wqkv[cb * 128:(cb + 1) * 128, 512:1024]))
        for cb in range(CBN):
            loads.append((w_sb[cb][:, 1024:1536],
                          wqkv[cb * 128:(cb + 1) * 128, 1024:1536]))
        # segment 1's xT slice loads ahead of the seg-2/3 remainder so
        # window 0's drip (which starts while the load stream is still
        # draining) never stalls on it
        for cb in range(CBN):
            loads.append((xt_sb[cb][:, 512:1024],
                          xT[cb * 128:(cb + 1) * 128, 512:1024]))
        loads.append((padb_sb, padb))
        loads.append((tri_sb, tri))
        for cb in range(CBN):
            loads.append((xt_sb[cb][:, 1024:T],
                          xT[cb * 128:(cb + 1) * 128, 1024:T]))
        for c in range(4):
            loads.append((wo_sb[c], wout[c * 128:(c + 1) * 128, :]))
        for i, (dst, src) in enumerate(loads):
            qs[i % 2].dma_start(out=dst, in_=src)

        # q^T/k^T: [within-pair col (head-lo d / head-hi d), s-block, pair, t]
        qT_all = singles.tile([128, TB, NP, 128], bf, name="qT_all", tag="qT_all")
        kT_all = singles.tile([128, TB, NP, 128], bf, name="kT_all", tag="kT_all")
        # v with 64 ones columns per head: the AV matmul then emits the
        # softmax denominator replicated on 64 partitions (rows 64-127),
        # so the reciprocal runs wide instead of a 1-partition crawl
        vones = singles.tile([128, TB, HG, 128], bf, name="vones", tag="vones")
        # only window 0's ones blocks up front (the full-tensor memset was
        # 6.9us of DVE squarely on the startup critical chain); each later
        # segment memsets its own blocks as part of the drip
        nc.vector.memset(vones[:, 0:4, :, D:128], 1.0)

        # qkv + attention interleaved per 512-wide t-window so the PE stream
        # stays dense (HAM stays at 2.4 GHz): attention for window I only
        # needs q/k/v blocks 0..4I+3, which segment I of the qkv loop topped
        # off.  One shared PSUM pool: qkv 2 + sAB 2x2 + psT 2 = 8 banks.
        with tc.tile_pool(name="psum", bufs=2, space="PSUM") as psum, \
             tc.tile_pool(name="rope", bufs=4) as rope_pool, \
             tc.tile_pool(name="qknat", bufs=3) as qk_pool, \
             tc.tile_pool(name="exps", bufs=3) as exp_pool, \
             tc.tile_pool(name="attnT", bufs=4) as aT_pool, \
             tc.tile_pool(name="recips", bufs=2) as rc_pool, \
             tc.tile_pool(name="outsb", bufs=2) as out_pool:
            def qkv_segment(I):
                """Generator emitting segment I's qkv matmuls in half-tensor
                chunks (yield points), so the caller can drip them into the
                attention loop as PE gap-filler while ACT grinds exps."""
                if I > 0:
                    nc.vector.memset(vones[:, 4 * I:4 * I + 4, :, D:128], 1.0)
                # Segment 0 runs before the attention windows, so the sAB/
                # psT PSUM banks are still free: cycle its qkv tiles across
                # them for a 4-deep pipeline (the 2-buffer qkv rotation
                # otherwise WAR-stalls the PE behind DVE's rope reads of the
                # previous-but-one unit).  Its v-copies go to the then-idle
                # ACT engine for the same reason.
                tags = ("qkv", "avA", "qkv", "avB") if I == 0 else ("qkv",)
                u = 0
                for tb in range(4 * I, 4 * I + 4):
                    for which, base in (("q", 0), ("k", 512), ("v", 1024)):
                        tag = tags[u % len(tags)]
                        u += 1
                        ps = psum.tile([128, HG, D], f32, name=f"ps{which}",
                                       tag=tag, bufs=(2 if tag == "qkv" else 1))
                        for cb in range(CBN):
                            nc.tensor.matmul(
                                ps, xt_sb[cb][:, tb * 128:(tb + 1) * 128],
                                w_sb[cb][:, base:base + 512],
                                start=(cb == 0), stop=(cb == CBN - 1))
                            if cb == 3:
                                yield
                        if which == "v":
                            if I == 0:
                                nc.scalar.activation(
                                    out=vones[:, tb, :, 0:D], in_=ps, func=CPY)
                            else:
                                nc.vector.tensor_copy(
                                    out=vones[:, tb, :, 0:D], in_=ps)
                            yield
                            continue
                        # rope: P_c = qkv*cosF, P_s = qkv*sinF (compact [T,64]
                        # tables broadcast over the 8 heads; both halves of
                        # each head carry the same table value), then
                        # lo = P_c.lo - P_s.hi ; hi = P_s.lo + P_c.hi
                        cosb = cos_sb[:, tb].unsqueeze(1).broadcast_to((128, HG, D))
                        sinb = sin_sb[:, tb].unsqueeze(1).broadcast_to((128, HG, D))
                        pc = rope_pool.tile([128, HG, D], f32, name="pc", tag="rt")
                        psn = rope_pool.tile([128, HG, D], f32, name="psn", tag="rt")
                        nc.vector.tensor_mul(pc, ps, cosb)
                        nc.vector.tensor_mul(psn, ps, sinb)
                        ro = qk_pool.tile([128, HG, D], bf, name="ro", tag="ro")
                        nc.vector.tensor_sub(
                            ro[:, :, 0:32], pc[:, :, 0:32], psn[:, :, 32:64])
                        nc.vector.tensor_add(
                            ro[:, :, 32:64], psn[:, :, 0:32], pc[:, :, 32:64])
                        dst = qT_all if which == "q" else kT_all
                        nc.sync.dma_start_transpose(out=dst[:, tb, :, :], in_=ro)
                        yield

            pending_out = []
            for I in range(NI):
                if I == 0:
                    for _ in qkv_segment(0):
                        pass
                nxt = qkv_segment(I + 1) if I + 1 < NI else None
                n_chunks = 4 * 3 * 3  # yield points per segment
                n_iters = 4 * (4 * I + 4)
                emitted = it = 0
                # out-projection pop budget per window: defer most of it into
                # the late, ACT-bound windows where the PE otherwise idles
                # behind the exp stream
                pops_left = (0, 4, 8, 10 ** 9)[I]

                def drip():
                    nonlocal emitted
                    got = 0
                    if nxt is None:
                        return got
                    if I == 0:
                        # window 0 overlaps the tail of the input-load
                        # stream: spread segment 1 across the whole window
                        # instead of front-cramming it into the load-starved
                        # first half
                        due = (it * n_chunks) // (n_iters - 2)
                    else:
                        # finish the segment by ~75% of the window so the
                        # next window's S matmuls never wait on transposes
                        due = (4 * it * n_chunks) // (3 * n_iters)
                    while emitted < due:
                        if next(nxt, "done") == "done":
                            break
                        emitted += 1
                        got += 1
                    return got

                # ---- attention window I ----
                aT_I = aT_pool.tile([128, NP, 512], bf, name="aT_I", tag="aT_I")
                for p in range(NP):
                    psTA = psum.tile([128, 512], f32, name="psTA", tag="avA", bufs=1)
                    psTB = psum.tile([128, 512], f32, name="psTB", tag="avB", bufs=1)

                    def emit_av(j, eAB):
                        off = max(j - 4 * I, 0) * 128
                        for h2, psT in ((0, psTA), (1, psTB)):
                            nc.tensor.matmul(
                                psT[:, off:512],
                                vones[:, j, 2 * p + h2, :],
                                eAB[:, h2, off:512],
                                start=(j == 0), stop=(j == 4 * I + 3))

                    prev = None
                    dry = False
                    for j in range(4 * I + 4):
                        jl = j - 4 * I
                        off = max(jl, 0) * 128
                        sAB = psum.tile([128, 2, 512], f32, name="sAB", tag="sAB", bufs=2)
                        if dry:
                            # "heater" matmul: PE would otherwise sit ~50%
                            # idle behind ACT and HAM-downclock to 1.2 GHz;
                            # burn a throwaway matmul into the bank the next
                            # S matmul overwrites anyway
                            nc.tensor.matmul(
                                sAB[:, 0, :], xt_sb[0][:, 0:128], xt_sb[0][:, 0:512],
                                start=True, stop=True, skip_group_check=True)
                        HALVES = ((0, slice(0, 64)), (1, slice(64, 128)))
                        if jl >= 0:
                            # keep the row-tiled pair adjacent so the two
                            # heads overlap in the PE array
                            for h2, rows in HALVES:
                                nc.tensor.matmul(
                                    sAB[:, h2, off:off + 128],
                                    kT_all[rows, j, p, :],
                                    qT_all[rows, 4 * I + jl, p, :],
                                    start=True, stop=True,
                                    tile_position=(h2 * 64, 0),
                                    skip_group_check=True)
                            if off + 128 < 512:
                                for h2, rows in HALVES:
                                    nc.tensor.matmul(
                                        sAB[:, h2, off + 128:512],
                                        kT_all[rows, j, p, :],
                                        qT_all[rows, 4 * I + jl + 1:4 * I + 4, p, :],
                                        start=True, stop=True,
                                        tile_position=(h2 * 64, 0))
                        else:
                            for h2, rows in HALVES:
                                nc.tensor.matmul(
                                    sAB[:, h2, :],
                                    kT_all[rows, j, p, :],
                                    qT_all[rows, 4 * I:4 * I + 4, p, :],
                                    start=True, stop=True,
                                    tile_position=(h2 * 64, 0))
                        eAB = exp_pool.tile([128, 2, 512], bf, name="eAB", tag="eAB")
                        bias = padb_sb[:, j:j + 1]
                        nc.scalar.activation(
                            out=eAB[:, :, off:512], in_=sAB[:, :, off:512],
                            func=EXP, bias=bias, scale=0.125)
                        if jl >= 0:
                            # zero the diagonal block's upper triangle (keys
                            # after the query) post-exp: bf16 2x-mode DVE
                            # multiply with the 0/1 mask, replacing the PSUM
                            # -1e30 seeding matmuls
                            trib = tri_sb.unsqueeze(1).broadcast_to((128, 2, 128))
                            nc.vector.tensor_mul(
                                eAB[:, :, off:off + 128],
                                eAB[:, :, off:off + 128], trib)
                        if prev is not None:
                            emit_av(*prev)
                        prev = (j, eAB)
                        it += 1
                        did_out = False
                        if pending_out and pops_left > 0:
                            pending_out.pop(0)()
                            pops_left -= 1
                            did_out = True
                        dry = drip() == 0 and not did_out and (it % 2 == 0)
                    emit_av(*prev)

                    # softmax normalization: stage both heads' replicated
                    # denominators (psT[64:128]) into one SBUF tile, one
                    # reciprocal_approx_fast over [128,512] (custom DVE op,
                    # ~5x faster than the iterative divide -- but it reads
                    # matmul-written PSUM as garbage, hence the SBUF staging
                    # copies), then multiply the numerators straight off
                    # PSUM into the out-projection's stationary layout.
                    cpd = rc_pool.tile([128, 512], f32, name="cpd", tag="cpd")
                    nc.vector.tensor_copy(out=cpd[0:64, :], in_=psTA[D:128, :])
                    nc.vector.tensor_copy(out=cpd[64:128, :], in_=psTB[D:128, :])
                    rc = rc_pool.tile([128, 512], f32, name="rc", tag="rc")
                    nc.vector.reciprocal_approx_fast(rc, cpd)
                    nc.vector.tensor_mul(
                        aT_I[0:64, p, :], psTA[0:D, :], rc[0:64, :])
                    nc.vector.tensor_mul(
                        aT_I[64:128, p, :], psTB[0:D, :], rc[64:128, :])

                # out-projection units are deferred into the next window's
                # loop as more PE gap-filler
                def make_out_unit(aT, i, il, n):
                    def emit():
                        pso = psum.tile([128, 512], f32, name="pso", tag="sAB", bufs=2)
                        for c in range(4):
                            nc.tensor.matmul(
                                pso,
                                aT[:, c, il * 128:(il + 1) * 128],
                                wo_sb[c][:, n * 512:(n + 1) * 512],
                                start=(c == 0), stop=(c == 3))
                        osb = out_pool.tile([128, 512], bf, name="osb", tag="osb")
                        nc.vector.tensor_copy(out=osb, in_=pso)
                        nc.gpsimd.dma_start(
                            out=outp[i * 128:(i + 1) * 128, n * 512:(n + 1) * 512],
                            in_=osb)
                    return emit
                for il in range(4):
                    for n in range(2):
                        pending_out.append(make_out_unit(aT_I, 4 * I + il, il, n))
                if nxt is not None:
                    for _ in nxt:
                        pass
            for f in pending_out:
                f()

    nc.compile()
    return nc


def _get_program():
    global _PROG
    if _PROG is None:
        _PROG = _build_program()
    return _PROG


def _rope_tables():
    bf16 = ml_dtypes.bfloat16
    inv = 1.0 / (ROPE_BASE ** (np.arange(0, D, 2, dtype=np.float64) / D))
    f = np.arange(T, dtype=np.float64)[:, None] * inv[None, :]  # [T, 32]
    c = np.cos(f)
    s = np.sin(f)
    # both 32-col halves carry the same table value
    cosT = np.concatenate([c, c], axis=1).astype(bf16)  # [T, 64]
    sinT = np.concatenate([s, s], axis=1).astype(bf16)
    return cosT, sinT


def kernel(x, attention_mask, W_qkv, W_out, b_out):
    global _LAST_RESULTS
    from concourse.bass_utils import run_bass_kernel_spmd

    nc = _get_program()
    bf16 = ml_dtypes.bfloat16
    x = np.asarray(x, dtype=np.float32)
    attention_mask = np.asarray(attention_mask)
    W_qkv = np.asarray(W_qkv, dtype=np.float32)
    W_out = np.asarray(W_out, dtype=np.float32)
    b_out = np.asarray(b_out, dtype=np.float32)

    cosT, sinT = _rope_tables()
    # causal mask for the diagonal block: keep key s <= query t
    tri = np.where(np.arange(128)[:, None] <= np.arange(128)[None, :], 1.0, 0.0)
    tri = tri.astype(bf16)

    in_maps = []
    for g in range(NCORES):
        b, hg = g // 2, g % 2
        sl = slice(hg * 512, hg * 512 + 512)
        wq = W_qkv[:, 0 * C:][:, sl]
        wk = W_qkv[:, 1 * C:2 * C][:, sl]
        wv = W_qkv[:, 2 * C:3 * C][:, sl]
        wqkv_g = np.ascontiguousarray(
            np.concatenate([wq, wk, wv], axis=1)).astype(bf16)
        xT_g = np.ascontiguousarray(x[b].T).astype(bf16)
        wout_g = np.ascontiguousarray(W_out[sl, :]).astype(bf16)
        padb_g = np.ascontiguousarray(
            np.where(attention_mask[b] != 0, 0.0, -1e30)
            .astype(np.float32).reshape(TB, 128).T)
        in_maps.append({
            "xT": xT_g, "wqkv": wqkv_g, "wout": wout_g,
            "cosT": cosT, "sinT": sinT, "padb": padb_g, "tri": tri,
        })

    res = run_bass_kernel_spmd(nc, in_maps, list(range(NCORES)))
    _LAST_RESULTS = res
    out = np.empty((B, T, C), dtype=np.float32)
    for b in range(B):
        out[b] = (res.results[2 * b]["outp"].astype(np.float32)
                  + res.results[2 * b + 1]["outp"].astype(np.float32) + b_out)
    return out
